# revision 13
# baseline (speedup 1.0000x reference)
"""Trainium2 Bass kernel for nn_DepGraph (relaxed-Bernoulli dependency-graph sampling).

Computes, for fixed N=M=4096, d=256:
  G = unsort(triu_sample(pairwise_logits(Y, Y), u_G)),  Y = uR[argsort(log_cdf(uR))]
  A = sample(pairwise_logits(uM, uR), u_A)
returns np.stack([G, A]).

Math restructure used on device (per element, z = -0.5*d2/scale <= 0):
  logitexp(z)  == -log(expm1(-z))
  sample(logit, u) = sigmoid((logit + log u - log(1-u))/T)
                   = 1 / (1 + w**(1/T)),  w = expm1(-z) * (1-u)/u
so per element we need ACT {Exp, Ln, Exp} (all in one ACT table set) and a few
DVE ops incl. reciprocal_approx_fast.  d2 row-blocks come from fp32r matmuls;
the +r_i +r_j (+mask bias) terms are folded in via a rank-2 epilogue matmul
with lhsT rows [r_i; 1] and rhs rows [1; rY + bias_slot].  Rows are sharded
8 ways (512 rows/core, SPMD); the strict-upper-triangle masking of G adds BIG
to d2 for all columns j < 128*(R+1) (drives the sample to ~0 = below-threshold
of fp32), and the 128x128 diagonal blocks are recomputed separately with an
exact strict-upper mask.  Row sort/unsort is index bookkeeping done on host
(mirrors the reference's eager fp32 jax computation bit-exactly).
"""

import os
import numpy as np

# ---------------------------------------------------------------- constants
N = 4096
D = 256
P = 128
NCORES = 8
RPC = N // NCORES          # rows per core = 512
SLOTS = RPC // P           # 128-row slots per core = 4
WHALF = 1024               # columns per psum/ACT/DVE unit
WDVE = 1024
TEMPERATURE = 0.3
EPS = 1e-6
BIG = 4000.0               # added to d2 to kill masked columns
HI = 1e11                  # clamp on w so that w**(1/T) stays finite in fp32
LO = 1e-30                 # lower clamp (diag blocks only)

f32 = np.float32

_PROGRAM_CACHE = {}
LAST_RESULTS = None        # test harness can inspect exec_time_ns etc.


def _sort_indices(uR: np.ndarray) -> np.ndarray:
    """Mirror of the reference's order statistic, computed eagerly on CPU jax
    (bit-exact with `reference()` called un-jitted)."""
    import jax
    import jax.numpy as jnp

    cpu = jax.devices("cpu")[0]
    with jax.default_device(cpu):
        x = jnp.asarray(np.ascontiguousarray(uR))
        log_cdf = jnp.sum(jnp.log(0.5 + 0.5 * jax.lax.erf(x / np.sqrt(2.0))), axis=1)
        si = jnp.argsort(log_cdf)
        return np.asarray(si)


def _build_program(n=N, ncores=NCORES, whalf=WHALF, wdve=WDVE, d=D):
    """Build the SPMD Bass/Tile program (shared by all 8 cores)."""
    import concourse.bass as bass
    import concourse.bacc as bacc
    import concourse.mybir as mybir
    from concourse import tile

    dt = mybir.dt
    AF = mybir.ActivationFunctionType
    OP = mybir.AluOpType
    F32 = dt.float32
    F32R = dt.float32r

    c_exp = float(f32(0.5) / f32(np.exp(0.5 * np.log(d))))   # 0.5/scale = 1/32
    inv_t = float(f32(1.0) / f32(TEMPERATURE))

    rpc = n // ncores
    slots = rpc // P
    nc = bacc.Bacc(None, target_bir_lowering=False)

    # ---------------- DRAM I/O (shapes identical on every core) ----------
    d_yt = [nc.dram_tensor(f"yt{k}", [P, n], F32R, kind="ExternalInput") for k in range(2)]
    d_urt = [nc.dram_tensor(f"urt{k}", [P, n], F32R, kind="ExternalInput") for k in range(2)]
    d_lhsG = nc.dram_tensor("lhsG", [2, P, rpc], F32R, kind="ExternalInput")
    d_lhsA = nc.dram_tensor("lhsA", [2, P, rpc], F32R, kind="ExternalInput")
    d_r2g = nc.dram_tensor("r2g", [2, n], F32, kind="ExternalInput")       # [ones; rY]
    d_r2a = nc.dram_tensor("r2a", [2, n], F32, kind="ExternalInput")       # [ones; rR]
    d_l2g = nc.dram_tensor("l2g", [slots, 2, P], F32, kind="ExternalInput")  # [rY_rows; 1] per slot
    d_l2a = nc.dram_tensor("l2a", [slots, 2, P], F32, kind="ExternalInput")  # [rM_rows; 1] per slot
    d_rdg = nc.dram_tensor("rdg", [slots, 2, P], F32, kind="ExternalInput")  # [1; rY_diag] per slot
    d_uG = nc.dram_tensor("uG", [rpc, n], F32, kind="ExternalInput")
    d_uA = nc.dram_tensor("uA", [rpc, n], F32, kind="ExternalInput")
    d_ytd = nc.dram_tensor("ytd", [slots, 2, P, P], F32R, kind="ExternalInput")
    d_uGd = nc.dram_tensor("uGd", [slots, P, P], F32, kind="ExternalInput")
    d_triu = nc.dram_tensor("triu", [P, P], F32, kind="ExternalInput")
    d_outG = nc.dram_tensor("outG", [rpc, n], F32, kind="ExternalOutput")
    d_outA = nc.dram_tensor("outA", [rpc, n], F32, kind="ExternalOutput")
    d_outGd = nc.dram_tensor("outGd", [slots, P, P], F32, kind="ExternalOutput")

    with tile.TileContext(nc) as tc:
        with (
            tc.tile_pool(name="const", bufs=1) as const,
            tc.tile_pool(name="upool", bufs=3) as upool,
            tc.tile_pool(name="scr", bufs=12) as scr,
            tc.tile_pool(name="spool", bufs=2) as spool,
            tc.tile_pool(name="psum", bufs=2, space="PSUM") as psum_pool,
            tc.tile_pool(name="psumd", bufs=2, space="PSUM") as psumd_pool,
        ):
            # ---------------- resident constants ----------------
            t_yt, t_urt, t_lhsG, t_lhsA = [], [], [], []
            for k in range(2):
                t = const.tile([P, n], F32R, tag=f"yt{k}")
                nc.sync.dma_start(t[:], d_yt[k][:])
                t_yt.append(t)
                t = const.tile([P, n], F32R, tag=f"urt{k}")
                nc.sync.dma_start(t[:], d_urt[k][:])
                t_urt.append(t)
                t = const.tile([P, rpc], F32R, tag=f"lhsG{k}")
                nc.sync.dma_start(t[:], d_lhsG[k])
                t_lhsG.append(t)
                t = const.tile([P, rpc], F32R, tag=f"lhsA{k}")
                nc.sync.dma_start(t[:], d_lhsA[k])
                t_lhsA.append(t)
            t_r2g = const.tile([2, n], F32, tag="r2g")
            nc.sync.dma_start(t_r2g[:], d_r2g[:])
            t_r2a = const.tile([2, n], F32, tag="r2a")
            nc.sync.dma_start(t_r2a[:], d_r2a[:])
            t_l2g, t_l2a, t_rdg = [], [], []
            for s in range(slots):
                t = const.tile([2, P], F32, tag=f"l2g{s}")
                nc.sync.dma_start(t[:], d_l2g[s])
                t_l2g.append(t)
                t = const.tile([2, P], F32, tag=f"l2a{s}")
                nc.sync.dma_start(t[:], d_l2a[s])
                t_l2a.append(t)
                t = const.tile([2, P], F32, tag=f"rdg{s}")
                nc.sync.dma_start(t[:], d_rdg[s])
                t_rdg.append(t)
            t_ytd = []
            for s in range(slots):
                pair = []
                for k in range(2):
                    t = const.tile([P, P], F32R, tag=f"ytd{s}_{k}")
                    nc.sync.dma_start(t[:], d_ytd[s, k])
                    pair.append(t)
                t_ytd.append(pair)
            t_triu = const.tile([P, P], F32, tag="triu")
            nc.sync.dma_start(t_triu[:], d_triu[:])




            def elementwise(e2, u_src_ap, out_ap, width, diag_mask=None):
                """u -> s given e2 = exp(c*d2); writes s (width cols) to out_ap."""
                u_t = upool.tile([P, width], F32, tag="u")
                nc.sync.dma_start(u_t[:], u_src_ap)
                cu = scr.tile([P, width], F32, tag="scr")
                nc.gpsimd.tensor_scalar(cu[:], u_t[:], float(f32(1.0) - f32(EPS)), float(EPS), OP.min, OP.max)
                r = scr.tile([P, width], F32, tag="scr")
                nc.vector.reciprocal_approx_fast(r[:], cu[:])
                omu = scr.tile([P, width], F32, tag="scr")
                nc.gpsimd.tensor_scalar(omu[:], cu[:], -1.0, 1.0, OP.mult, OP.add)
                q = scr.tile([P, width], F32, tag="scr")
                nc.vector.tensor_tensor(q[:], omu[:], r[:], OP.mult)
                w1 = scr.tile([P, width], F32, tag="scr")
                nc.vector.tensor_tensor(w1[:], e2[:], q[:], OP.mult)
                w2 = scr.tile([P, width], F32, tag="scr")
                nc.vector.tensor_tensor(w2[:], w1[:], q[:], OP.subtract)
                wc = scr.tile([P, width], F32, tag="scr")
                nc.vector.tensor_scalar(wc[:], w2[:], float(LO), float(HI), OP.max, OP.min)
                lw = scr.tile([P, width], F32, tag="scr")
                nc.scalar.activation(lw[:], wc[:], AF.Ln)
                pw = scr.tile([P, width], F32, tag="scr")
                nc.scalar.activation(pw[:], lw[:], AF.Exp, scale=inv_t)
                p1 = scr.tile([P, width], F32, tag="scr")
                nc.gpsimd.tensor_scalar(p1[:], pw[:], 1.0, None, OP.add)
                s_t = spool.tile([P, width], F32, tag="s")
                nc.vector.reciprocal_approx_fast(s_t[:], p1[:])
                if diag_mask is not None:
                    sm = spool.tile([P, width], F32, tag="sm")
                    nc.vector.tensor_tensor(sm[:], s_t[:], diag_mask[:], OP.mult)
                    s_t = sm
                nc.sync.dma_start(out_ap, s_t[:])

            # ---------------- main units ----------------
            for slot in range(slots):
                rows = slice(slot * P, (slot + 1) * P)
                for mat in range(2):  # 0 = G, 1 = A
                    lhs = t_lhsG if mat == 0 else t_lhsA
                    rhs = t_yt if mat == 0 else t_urt
                    l2 = t_l2g[slot][:] if mat == 0 else t_l2a[slot][:]
                    d_u = d_uG if mat == 0 else d_uA
                    d_out = d_outG if mat == 0 else d_outA
                    for h in range(n // whalf):
                        pt = psum_pool.tile([P, whalf], F32, tag="ps")
                        for j in range(whalf // 512):
                            cols = slice(h * whalf + j * 512, h * whalf + (j + 1) * 512)
                            pcols = slice(j * 512, (j + 1) * 512)
                            nc.tensor.matmul(
                                pt[:, pcols],
                                lhs[0][:, rows],
                                rhs[0][:, cols],
                                start=True, stop=False,
                            )
                            nc.tensor.matmul(
                                pt[:, pcols],
                                lhs[1][:, rows],
                                rhs[1][:, cols],
                                start=False, stop=False,
                            )
                            r2 = (t_r2g if mat == 0 else t_r2a)[:, cols]
                            nc.tensor.matmul(pt[:, pcols], l2, r2, start=False, stop=True)
                        e2 = scr.tile([P, whalf], F32, tag="scr")
                        nc.scalar.activation(e2[:], pt[:], AF.Exp, scale=c_exp)
                        elementwise(
                            e2,
                            d_u[rows, h * whalf:(h + 1) * whalf],
                            d_out[rows, h * whalf:(h + 1) * whalf],
                            whalf,
                        )

            # ---------------- diagonal blocks of G ----------------
            for slot in range(slots):
                rows = slice(slot * P, (slot + 1) * P)
                pt = psumd_pool.tile([P, P], F32, tag="psd")
                nc.tensor.matmul(pt[:], t_lhsG[0][:, rows],
                                 t_ytd[slot][0][:], start=True, stop=False)
                nc.tensor.matmul(pt[:], t_lhsG[1][:, rows],
                                 t_ytd[slot][1][:], start=False, stop=False)
                nc.tensor.matmul(pt[:], t_l2g[slot][:], t_rdg[slot][:],
                                 start=False, stop=True)
                e2 = scr.tile([P, P], F32, tag="scrd")
                nc.scalar.activation(e2[:], pt[:], AF.Exp, scale=c_exp)
                elementwise(e2, d_uGd[slot], d_outGd[slot], P, diag_mask=t_triu)

    nc.finalize()
    return nc


def _get_program():
    if "nc" not in _PROGRAM_CACHE:
        _PROGRAM_CACHE["nc"] = _build_program()
    return _PROGRAM_CACHE["nc"]


def _host_prep(uR, uM, u_G, u_A, si, n=N, ncores=NCORES):
    """Build per-core input maps (shared between kernel() and tests)."""
    rpc = n // ncores
    slots = rpc // P
    Y = np.ascontiguousarray(uR[si])
    YT = np.ascontiguousarray(Y.T)
    URT = np.ascontiguousarray(uR.T)
    YTm2 = np.ascontiguousarray((-2.0 * YT).astype(f32))
    UMTm2 = np.ascontiguousarray((-2.0 * uM.T).astype(f32))
    rY = (Y * Y).sum(axis=1, dtype=np.float32).astype(f32)
    rR = (uR * uR).sum(axis=1, dtype=np.float32).astype(f32)
    rM = (uM * uM).sum(axis=1, dtype=np.float32).astype(f32)
    ones = np.ones(n, dtype=f32)
    triu = np.triu(np.ones((P, P), dtype=f32), k=1)
    r2a = np.ascontiguousarray(np.stack([ones, rR]))
    r2g = np.ascontiguousarray(np.stack([ones, rY]))

    # Kill the masked (below/at diagonal-block) region of G by forcing u -> 0
    # there (clips to EPS on device => logistic = -13.8 => sample ~ 0).  The
    # true diagonal blocks are extracted into uGd before masking.
    uGd_all = np.empty((n // P, P, P), dtype=f32)
    for R in range(n // P):
        srows = slice(R * P, (R + 1) * P)
        uGd_all[R] = u_G[srows, srows]
    u_G_kill = u_G.copy()
    for R in range(n // P):
        u_G_kill[R * P:(R + 1) * P, : (R + 1) * P] = 0.0

    in_maps = []
    for c in range(ncores):
        rows = slice(c * rpc, (c + 1) * rpc)
        lhsG = np.ascontiguousarray(YTm2[:, rows].reshape(2, P, rpc))
        lhsA = np.ascontiguousarray(UMTm2[:, rows].reshape(2, P, rpc))
        l2g = np.empty((slots, 2, P), dtype=f32)
        l2a = np.empty((slots, 2, P), dtype=f32)
        rdg = np.empty((slots, 2, P), dtype=f32)
        ytd = np.empty((slots, 2, P, P), dtype=f32)
        uGd = np.empty((slots, P, P), dtype=f32)
        for s in range(slots):
            R = c * slots + s
            srows = slice(R * P, (R + 1) * P)
            l2g[s, 0] = rY[srows]; l2g[s, 1] = 1.0
            l2a[s, 0] = rM[srows]; l2a[s, 1] = 1.0
            rdg[s, 0] = 1.0; rdg[s, 1] = rY[srows]
            ytd[s] = YT[:, srows].reshape(2, P, P)
            uGd[s] = uGd_all[R]
        in_maps.append({
            "yt0": np.ascontiguousarray(YT[:P]),
            "yt1": np.ascontiguousarray(YT[P:]),
            "urt0": np.ascontiguousarray(URT[:P]),
            "urt1": np.ascontiguousarray(URT[P:]),
            "lhsG": lhsG, "lhsA": lhsA,
            "r2g": r2g, "r2a": r2a, "l2g": l2g, "l2a": l2a, "rdg": rdg,
            "uG": np.ascontiguousarray(u_G_kill[rows]),
            "uA": np.ascontiguousarray(u_A[rows]),
            "ytd": ytd, "uGd": uGd, "triu": triu,
        })
    return in_maps


def kernel(uR, uM, g_logscale, u_G, u_A):
    global LAST_RESULTS
    from concourse import bass_utils

    uR = np.ascontiguousarray(np.asarray(uR, dtype=f32))
    uM = np.ascontiguousarray(np.asarray(uM, dtype=f32))
    u_G = np.ascontiguousarray(np.asarray(u_G, dtype=f32))
    u_A = np.ascontiguousarray(np.asarray(u_A, dtype=f32))

    si = _sort_indices(uR)
    inv = np.argsort(si, kind="stable")
    in_maps = _host_prep(uR, uM, u_G, u_A, si)

    nc = _get_program()
    trace = os.environ.get("DEPGRAPH_TRACE", "") == "1"
    res = bass_utils.run_bass_kernel_spmd(
        nc, in_maps, core_ids=list(range(NCORES)), trace=trace,
    )
    LAST_RESULTS = res

    Gs = np.empty((N, N), dtype=f32)
    A = np.empty((N, N), dtype=f32)
    for c in range(NCORES):
        rows = slice(c * RPC, (c + 1) * RPC)
        Gs[rows] = res.results[c]["outG"]
        A[rows] = res.results[c]["outA"]
        for s in range(SLOTS):
            R = c * SLOTS + s
            srows = slice(R * P, (R + 1) * P)
            Gs[srows, srows] = res.results[c]["outGd"][s]
    G = Gs[inv][:, inv]
    return np.stack([G, A])
